# revision 2
# baseline (speedup 1.0000x reference)
"""Trainium2 Bass kernel for nn_BatchedDynamicThresholdLIF.

Reference (fp32): T=1000 sequential steps on state (B=64, N=1024):
    vp = A + x_t                  (A = decayed membrane carry)
    s  = f32(vp >= th)
    A' = select(s, -65, fl(fl(vp*0.95) - 3.25))
    th'= fl(fl(s*5) + fl(fl(th*0.99) - 0.5))
The A'/th' forms are algebraically equal to the XLA lowering of the
reference but rounded differently; measured against the bit-exact
emulation they flip 26 of 65.5M spikes (rel err 8.6e-3, gate 2e-2).

Sharding: data-parallel over B across 8 cores (8 batch rows per core =
8192 state elements, [128 partitions x 64 free]). The T recurrence is
local per core; no cross-core communication.

All six per-step instructions run on DVE: a single in-order engine has
no cross-engine semaphore hops, which dominated the 8-instruction
DVE/Pool split (1.51 ms). Instruction order separates each RAW pair by
at least one independent instruction where possible.
"""
import numpy as np

T, B, N = 1000, 64, 1024
NCORES = 8
BS = B // NCORES            # batch rows per core
S = BS * N                  # 8192 state elements per core
P = 128                     # SBUF partitions
F = S // P                  # 64 free elements per partition
KB = 50                     # timesteps per DMA block

_nc_cache = {}


def _build():
    import concourse.bacc as bacc
    import concourse.mybir as mybir
    import concourse.tile as tile

    f32 = mybir.dt.float32
    A = mybir.AluOpType
    nc = bacc.Bacc(None)
    x = nc.dram_tensor("x", [T, S], f32, kind="ExternalInput")
    so = nc.dram_tensor("s", [T, S], f32, kind="ExternalOutput")
    xv = x.rearrange("t (p j) -> p t j", p=P)
    sv = so.rearrange("t (p j) -> p t j", p=P)
    nblk = T // KB

    with tile.TileContext(nc) as tc:
        with tc.tile_pool(name="st", bufs=1) as stp, \
             tc.tile_pool(name="scr", bufs=2) as scr, \
             tc.tile_pool(name="xp", bufs=3) as xp, \
             tc.tile_pool(name="sp", bufs=3) as sp:
            av = stp.tile([P, F], f32, name="av")
            th = stp.tile([P, F], f32, name="th")
            neg65 = stp.tile([P, F], f32, name="neg65")
            nc.vector.memset(av, -65.0)
            nc.vector.memset(neg65, -65.0)
            nc.vector.memset(th, -50.0)
            for b in range(nblk):
                xb = xp.tile([P, KB, F], f32, name="xb", tag="xb")
                nc.sync.dma_start(out=xb, in_=xv[:, b * KB:(b + 1) * KB, :])
                sb = sp.tile([P, KB, F], f32, name="sb", tag="sb")
                for k in range(KB):
                    xt = xb[:, k, :]
                    st_ = sb[:, k, :]
                    vp = scr.tile([P, F], f32, name="vp", tag="vp")
                    m = scr.tile([P, F], f32, name="m", tag="m")
                    nc.vector.tensor_tensor(vp, av, xt, A.add)
                    nc.vector.tensor_scalar(m, th, 0.99, 0.5, A.mult, A.subtract)
                    nc.vector.tensor_tensor(st_, vp, th, A.is_ge)
                    nc.vector.tensor_scalar(av, vp, 0.95, 3.25, A.mult, A.subtract)
                    nc.vector.scalar_tensor_tensor(th, st_, 5.0, m, A.mult, A.add)
                    nc.vector.copy_predicated(
                        av, st_.bitcast(mybir.dt.uint32), neg65)
                nc.sync.dma_start(out=sv[:, b * KB:(b + 1) * KB, :], in_=sb)
    nc.compile()
    return nc


def _get_nc():
    if "nc" not in _nc_cache:
        _nc_cache["nc"] = _build()
    return _nc_cache["nc"]


def kernel(weighted_input: np.ndarray) -> np.ndarray:
    from concourse.bass_utils import run_bass_kernel_spmd

    x = np.ascontiguousarray(np.asarray(weighted_input, dtype=np.float32))
    assert x.shape == (T, B, N), x.shape
    nc = _get_nc()
    in_maps = []
    for c in range(NCORES):
        xc = np.ascontiguousarray(x[:, c * BS:(c + 1) * BS, :].reshape(T, S))
        in_maps.append({"x": xc})
    res = run_bass_kernel_spmd(nc, in_maps, core_ids=list(range(NCORES)))
    out = np.empty((T, B, N), np.float32)
    for c in range(NCORES):
        out[:, c * BS:(c + 1) * BS, :] = res.results[c]["s"].reshape(T, BS, N)
    return out


if __name__ == "__main__":
    x = np.random.default_rng(0).standard_normal((T, B, N)).astype(np.float32) * 3.0
    s = kernel(x)
    print("spike rate:", s.mean())


# revision 3
# speedup vs baseline: 1.1564x; 1.1564x over previous
"""Trainium2 Bass kernel for nn_BatchedDynamicThresholdLIF.

Per step (fp32), on state (B=64, N=1024) for T=1000 steps:
    vp  = A + x_t                 (A = decayed membrane carry)
    s   = f32(vp >= th)
    A'  = select(vp >= th, -65, fl(fl(vp*0.95) - 3.25))
    th' = fl(fl(fl(th*0.99) - 0.5) + select(vp >= th, 5, 0))
Algebraically equal to the XLA lowering of the reference, rounded
slightly differently: flips 26 of 65.5M spikes vs the bit-exact
emulation (rel err 8.6e-3, gate 2e-2).

A'/th' each run as ONE custom-DVE instruction (registered at import
into concourse.dve_ops.OPS; the per-NEFF uop table is generated by the
normal compile path), so a step is 4 DVE instructions instead of 6
stock ones (which measured 955us) or the original 8-instruction
DVE/Pool split (1513us).

GROUPS splits the 64 free elements per partition into independent
neuron groups whose instruction streams interleave, hiding the ~260ns
DVE read-after-write latency that otherwise gates the serial chain.

Sharding: data-parallel over B across 8 cores (8 batch rows per core =
8192 state elements, [128 partitions x 64 free]); T recurrence local
per core, no cross-core communication.
"""
import numpy as np

T, B, N = 1000, 64, 1024
NCORES = 8
BS = B // NCORES            # batch rows per core
S = BS * N                  # 8192 state elements per core
P = 128                     # SBUF partitions
F = S // P                  # 64 free elements per partition
KB = 50                     # timesteps per DMA block
GROUPS = 1                  # independent interleaved neuron groups
FG = F // GROUPS

_nc_cache = {}


def _register_ops():
    import concourse.dve_ops as dve_ops
    from concourse.dve_spec import Spec, Src0, Src1, C0, C1, C2, Zero, select

    if "LIF_VRESET_ANT" in dve_ops._SUB_OPCODE_FOR_NAME:
        return
    F32 = np.float32

    def _vreset_ref(in0, in1, s0, s1, imm2):
        raw = (in0.astype(F32) * F32(s1)).astype(F32) + F32(imm2)
        return np.where(in0 >= in1, F32(s0), raw.astype(F32)).astype(F32)

    def _thresh_ref(in0, in1, s0, s1, imm2):
        m = ((in1.astype(F32) * F32(s0)).astype(F32) + F32(s1)).astype(F32)
        return (m + np.where(in0 >= in1, F32(imm2), F32(0))).astype(F32)

    ops = [
        dve_ops.DveOp(
            "LIF_VRESET_ANT",
            Spec(body=select(Src0 >= Src1, C0, Src0 * C1 + C2),
                 reference=_vreset_ref),
            subdim=False,
            uops_sha={"v3": "208ced3ffbf75254", "v4": "b110493593b247f2"},
        ),
        dve_ops.DveOp(
            "LIF_THRESH_ANT",
            Spec(body=(Src1 * C0 + C1) + select(Src0 >= Src1, C2, Zero),
                 reference=_thresh_ref),
            subdim=False,
            uops_sha={"v3": "c7541b824f2c4dca", "v4": "79a82a28adc320ad"},
        ),
    ]
    for op in ops:
        dve_ops.OPS.append(op)
        dve_ops._SUB_OPCODE_FOR_NAME[op.name] = (
            dve_ops._CUSTOM_DVE_ROW_BASE + len(dve_ops.OPS) - 1)
        dve_ops.CUSTOM_DVE_SPECS[op.name] = op.spec
    return


def _build():
    import concourse.bacc as bacc
    import concourse.mybir as mybir
    import concourse.tile as tile
    import concourse.dve_ops as dve_ops

    _register_ops()
    vreset = next(o for o in dve_ops.OPS if o.name == "LIF_VRESET_ANT")
    thresh = next(o for o in dve_ops.OPS if o.name == "LIF_THRESH_ANT")

    f32 = mybir.dt.float32
    A = mybir.AluOpType
    nc = bacc.Bacc(None)
    x = nc.dram_tensor("x", [T, S], f32, kind="ExternalInput")
    so = nc.dram_tensor("s", [T, S], f32, kind="ExternalOutput")
    xv = x.rearrange("t (p j) -> p t j", p=P)
    sv = so.rearrange("t (p j) -> p t j", p=P)
    nblk = T // KB
    G = GROUPS

    def gs(g):
        return slice(g * FG, (g + 1) * FG)

    with tile.TileContext(nc) as tc:
        with tc.tile_pool(name="st", bufs=1) as stp, \
             tc.tile_pool(name="scr", bufs=2) as scr, \
             tc.tile_pool(name="xp", bufs=3) as xp, \
             tc.tile_pool(name="sp", bufs=3) as sp:
            av = stp.tile([P, F], f32, name="av")
            th = stp.tile([P, F], f32, name="th")
            nc.vector.memset(av, -65.0)
            nc.vector.memset(th, -50.0)
            for b in range(nblk):
                xb = xp.tile([P, KB, F], f32, name="xb", tag="xb")
                nc.sync.dma_start(out=xb, in_=xv[:, b * KB:(b + 1) * KB, :])
                sb = sp.tile([P, KB, F], f32, name="sb", tag="sb")
                for k in range(KB):
                    vp = scr.tile([P, F], f32, name="vp", tag="vp")
                    for g in range(G):
                        nc.vector.tensor_tensor(
                            vp[:, gs(g)], av[:, gs(g)], xb[:, k, gs(g)], A.add)
                    for g in range(G):
                        nc.vector.tensor_tensor(
                            sb[:, k, gs(g)], vp[:, gs(g)], th[:, gs(g)], A.is_ge)
                    for g in range(G):
                        nc.vector._custom_dve(
                            vreset, out=av[:, gs(g)], in0=vp[:, gs(g)],
                            in1=th[:, gs(g)], s0=-65.0, s1=0.95, imm2=-3.25)
                    for g in range(G):
                        nc.vector._custom_dve(
                            thresh, out=th[:, gs(g)], in0=vp[:, gs(g)],
                            in1=th[:, gs(g)], s0=0.99, s1=-0.5, imm2=5.0)
                nc.sync.dma_start(out=sv[:, b * KB:(b + 1) * KB, :], in_=sb)
    nc.compile()
    return nc


def _get_nc():
    if "nc" not in _nc_cache:
        _nc_cache["nc"] = _build()
    return _nc_cache["nc"]


def kernel(weighted_input: np.ndarray) -> np.ndarray:
    from concourse.bass_utils import run_bass_kernel_spmd

    x = np.ascontiguousarray(np.asarray(weighted_input, dtype=np.float32))
    assert x.shape == (T, B, N), x.shape
    nc = _get_nc()
    in_maps = []
    for c in range(NCORES):
        xc = np.ascontiguousarray(x[:, c * BS:(c + 1) * BS, :].reshape(T, S))
        in_maps.append({"x": xc})
    res = run_bass_kernel_spmd(nc, in_maps, core_ids=list(range(NCORES)))
    out = np.empty((T, B, N), np.float32)
    for c in range(NCORES):
        out[:, c * BS:(c + 1) * BS, :] = res.results[c]["s"].reshape(T, BS, N)
    return out


if __name__ == "__main__":
    x = np.random.default_rng(0).standard_normal((T, B, N)).astype(np.float32) * 3.0
    s = kernel(x)
    print("spike rate:", s.mean())
